# revision 6
# baseline (speedup 1.0000x reference)
"""DePatchEmbed (patch scatter) for 8 trn2 NeuronCores.

Math (N=4, C=256, H=W=256, p=8, Gi=Gj=32, num_patches=1024, dim=16384):
    out[n, c, i*8+a, j*8+b] = x[n, j*32+i, c*64 + a*8 + b]

Sharding: core = n*2 + ch  (n = batch, ch = channel half).
  per-core input : x[n, :, ch*8192:(ch+1)*8192]          -> (1024, 8192) f32
  per-core output: out[n, ch*128:(ch+1)*128]             -> (128, 256, 256) f32
With the local channel c' on SBUF partitions, the permutation is purely
within-partition: per partition, free dim goes (j, a, b) -> (a, j, b).

Pipeline per i (row-block), 32 steps, 1:1 deps so DMAs carry single waits:
  load  (HWDGE/SP):  x4[:, i] = [c':128][j:32][e:64 contig]  (256 B chunks, 1 MB)
  shuffle (DVE):     tensor_copy (p, j, a, b) -> strides (8,256,1)
  store (HWDGE/ACT): o4[:, i] = [c':128][2048 contig]        (8 KB chunks, 1 MB)
"""

import numpy as np

import concourse.bacc as bacc
import concourse.bass as bass  # noqa: F401  (AP helpers)
import concourse.mybir as mybir
import concourse.tile as tile
from concourse.bass_utils import run_bass_kernel_spmd

N, C, H, W = 4, 256, 256, 256
P = 8          # patch size
GI = 32        # row blocks (i, scanned fastest in k)
GJ = 32        # col blocks (j)
DIM_HALF = 8192
G = 4          # i-values per pipeline step

_NC = None


def _build():
    # Bacc (not plain Bass): its finalize() runs generate_event_semaphores,
    # which splits multi-wait DMAs (HW allows 1 sync-wait per DMA trigger).
    nc = bacc.Bacc()
    x = nc.dram_tensor("x", [1024, DIM_HALF], mybir.dt.float32, kind="ExternalInput")
    o = nc.dram_tensor("o", [128, H, W], mybir.dt.float32, kind="ExternalOutput")
    # x4[c', i, j, e] with e = a*8+b : ap [[64,128],[8192*? see below],[...]]
    x4 = x.rearrange("(j i) (c e) -> c i j e", i=GI, c=128)  # (128, 32, 32, 64)
    o4 = o.rearrange("c (i r) w -> c i (r w)", r=P)          # (128, 32, 2048)

    with tile.TileContext(nc) as tc:
        with (
            tc.tile_pool(name="tin", bufs=6) as pin,
            tc.tile_pool(name="tout", bufs=6) as pout,
        ):
            for i in range(GI):
                t_in = pin.tile([128, 2048], mybir.dt.float32)
                t_out = pout.tile([128, 2048], mybir.dt.float32)
                # logical views (p, j, a, b); t_in laid out (j,a,b),
                # t_out laid out (a,j,b)
                tin4 = t_in[:].rearrange("p (j a b) -> p j a b", j=GJ, a=P)
                tout4 = t_out[:].rearrange("p (a j b) -> p j a b", a=P, j=GJ)
                nc.sync.dma_start(out=t_in[:], in_=x4[:, i])
                nc.vector.tensor_copy(out=tout4[:], in_=tin4[:])
                nc.scalar.dma_start(out=o4[:, i], in_=t_out[:])
    nc.finalize()
    return nc


def _get_nc():
    global _NC
    if _NC is None:
        _NC = _build()
    return _NC


def _shard_inputs(x_np):
    in_maps = []
    for c in range(8):
        n, ch = divmod(c, 2)
        in_maps.append(
            {"x": np.ascontiguousarray(x_np[n, :, ch * DIM_HALF : (ch + 1) * DIM_HALF])}
        )
    return in_maps


def _gather_outputs(results):
    out = np.empty((N, C, H, W), dtype=np.float32)
    for c in range(8):
        n, ch = divmod(c, 2)
        out[n, ch * 128 : (ch + 1) * 128] = results[c]["o"]
    return out


def run(x_np, **spmd_kwargs):
    """Run on 8 cores; returns (out, BassKernelResults)."""
    nc = _get_nc()
    res = run_bass_kernel_spmd(
        nc, _shard_inputs(x_np), core_ids=list(range(8)), **spmd_kwargs
    )
    return _gather_outputs(res.results), res


def kernel(x, ori_shape=None, patch_size=None, **_):
    x_np = np.asarray(x, dtype=np.float32).reshape(N, 1024, 2 * DIM_HALF)
    out, _res = run(x_np)
    return out


# revision 8
# speedup vs baseline: 1.3562x; 1.3562x over previous
"""DePatchEmbed (patch scatter) for 8 trn2 NeuronCores.

Math (N=4, C=256, H=W=256, p=8, Gi=Gj=32, num_patches=1024, dim=16384):
    out[n, c, i*8+a, j*8+b] = x[n, j*32+i, c*64 + a*8 + b]

Sharding: core = n*2 + ihalf  (n = batch, ihalf = half of the i row-blocks).
  per-core input : x[n].reshape(32,32,16384)[:, ihalf*16:(ihalf+1)*16, :]
                   -> x_core (j=32, i_loc=16, dim=16384)   32 MB contiguous
  per-core output: out[n, :, ihalf*128:(ihalf+1)*128, :]   -> (256, 128, 256)

Each SBUF partition p holds channels {2p, 2p+1}, so DRAM load chunks are
128 contiguous elements (512 B) and the permutation stays within-partition.

Pipeline per i_loc (16 steps):
  load  (HWDGE/SP):  [p:128][j:32][cc:128 contig]   512 B descs, 2 MB
  shuffle (DVE):     2 copies (c2=0,1): (p,j,a,b) strides (8,1)->(256,1),
                     j 128->8
  store (HWDGE/ACT): [p:128][c2:2][2048 contig]     8 KB descs, 2 MB
"""

import numpy as np

import concourse.bacc as bacc
import concourse.bass as bass  # noqa: F401
import concourse.mybir as mybir
import concourse.tile as tile
from concourse.bass_utils import run_bass_kernel_spmd

N, C, H, W = 4, 256, 256, 256
P = 8          # patch size
GI = 32        # row blocks (i, scanned fastest in k)
GJ = 32        # col blocks (j)
IL = 16        # i-values per core (GI / 2 halves)

_NC = None


def _build():
    # Bacc (not plain Bass): its finalize() runs generate_event_semaphores,
    # which splits multi-wait DMAs (HW allows 1 sync-wait per DMA trigger).
    nc = bacc.Bacc()
    x = nc.dram_tensor("x", [GJ, IL, 16384], mybir.dt.float32, kind="ExternalInput")
    o = nc.dram_tensor("o", [C, H // 2, W], mybir.dt.float32, kind="ExternalOutput")
    # x4[p, i_loc, j, cc]: ap [[128,128],[16384,16],[262144,32],[1,128]]
    x4 = x.rearrange("j i (p cc) -> p i j cc", p=128)
    # o4[p, i_loc, c2, rw]: ap [[65536,128],[2048,16],[32768,2],[1,2048]]
    o4 = o.rearrange("(p c2) (i r) w -> p i c2 (r w)", c2=2, r=P)

    with tile.TileContext(nc) as tc:
        with (
            tc.tile_pool(name="tin", bufs=4) as pin,
            tc.tile_pool(name="tout", bufs=4) as pout,
        ):
            for i in range(IL):
                t_in = pin.tile([128, 4096], mybir.dt.float32)
                t_out = pout.tile([128, 4096], mybir.dt.float32)
                # t_in laid out (j, c2, a, b); t_out laid out (c2, a, j, b)
                tin5 = t_in[:].rearrange("q (j c2 a b) -> q c2 j a b", c2=2, a=P, b=P)
                tout5 = t_out[:].rearrange(
                    "q (c2 a j b) -> q c2 j a b", c2=2, a=P, j=GJ, b=P
                )
                nc.sync.dma_start(out=t_in[:], in_=x4[:, i])
                for c2 in range(2):
                    nc.vector.tensor_copy(out=tout5[:, c2], in_=tin5[:, c2])
                nc.scalar.dma_start(out=o4[:, i], in_=t_out[:])
    nc.finalize()
    return nc


def _get_nc():
    global _NC
    if _NC is None:
        _NC = _build()
    return _NC


def _shard_inputs(x_np):
    v = x_np.reshape(N, GJ, GI, 16384)
    in_maps = []
    for core in range(8):
        n, ih = divmod(core, 2)
        in_maps.append(
            {"x": np.ascontiguousarray(v[n, :, ih * IL : (ih + 1) * IL, :])}
        )
    return in_maps


def _gather_outputs(results):
    out = np.empty((N, C, H, W), dtype=np.float32)
    for core in range(8):
        n, ih = divmod(core, 2)
        out[n, :, ih * 128 : (ih + 1) * 128, :] = results[core]["o"]
    return out


def run(x_np, **spmd_kwargs):
    """Run on 8 cores; returns (out, BassKernelResults)."""
    nc = _get_nc()
    res = run_bass_kernel_spmd(
        nc, _shard_inputs(x_np), core_ids=list(range(8)), **spmd_kwargs
    )
    return _gather_outputs(res.results), res


def kernel(x, ori_shape=None, patch_size=None, **_):
    x_np = np.asarray(x, dtype=np.float32).reshape(N, 1024, 16384)
    out, _res = run(x_np)
    return out
